# revision 38
# baseline (speedup 1.0000x reference)
"""Multi-head attention (B=2, S=2048, D=1024, H=16) on 8 NeuronCores.

Sharding: core c -> (batch b = c // 4, head-group g = c % 4, 4 heads each).
Each core computes its 4 heads' attention for its batch plus the partial
output projection (ctx_shard @ WO_shard.T).T; the host sums the 4 partials
per batch, adds the bias, and patches fully-masked query rows (where the
reference's softmax degenerates to uniform attention).

Device kernel layout notes:
  - x and the weight shards are pre-transposed on the host and fed as bf16.
  - Q,K are produced in [dk, s] layout (head-pair stacked on partitions) so
    scores come out transposed: S_t[k, q]. The two heads of a pair run as
    concurrent row-group matmuls (K=64 each).
  - Padding mask is applied by zeroing masked rows of V (and of the ones
    column), causal mask by multiplying the diagonal-band tiles with a
    precomputed 0/1 triangle strip.
  - Softmax normalization is deferred past the attention loop: V carries an
    extra ones column so P@V also accumulates row sums L[q]; unnormalized
    ctx and L are staged to SBUF, then a batched fast-reciprocal feeds the
    per-head 1/L rows back in: pair-0 and mid-run pair-1 tiles use a
    DRAM-bounce partition-broadcast hidden under later attention; the final
    tile uses a bf16 outer-product broadcast on the (idle) PE to cut the
    tail latency.
  - Per chunk, P@V is emitted one chunk behind scores/exp so every PE
    instruction in the in-order queue only depends on work finished at
    least one chunk earlier (no stalls, keeps the DVFS p-state high), and
    all projection matmuls (QKV both pairs + output projection) are queued
    as ~1-matmul "filler" units pumped into the exp-paced slots.
  - A run of warm-up matmuls holds the PE p-state up while the first x
    wave streams in; input DMA dispatches are spread over the Sync/GpSimd/
    Scalar sequencers; PSUM->SBUF copies are spread across Scalar/Vector.
"""

import os
import sys

import numpy as np

sys.path.insert(0, "/opt/trn_rl_repo")
os.environ.setdefault("MYCRO_LOCAL_CACHE", "1")

import ml_dtypes

import concourse.bass as bass
import concourse.tile as tile
from concourse import bacc, mybir
from concourse.bass_utils import run_bass_kernel_spmd

B, S, D, H = 2, 2048, 1024, 16
DK = D // H          # 64
NCORES = 8
HPC = H // (NCORES // B)   # heads per core = 4
DSH = HPC * DK             # 256: per-core shard of the model dim
NKC = S // 128             # 16 key chunks of 128
TRI_W = 384 + 512          # causal strip width

BF = mybir.dt.bfloat16
F32 = mybir.dt.float32
EXP = mybir.ActivationFunctionType.Exp

_NC_CACHE: list = []


class Filler:
    """Ordered queue of small emission units (~1 PE matmul each) pumped
    into the attention stream's PE gaps. require() force-drains up to a
    named mark so program order always respects data deps."""

    def __init__(self):
        self.q = []
        self.i = 0
        self.marks = {}

    def add(self, fn):
        self.q.append(fn)

    def mark(self, name):
        self.marks[name] = len(self.q)

    def pump(self, n):
        n = min(n, len(self.q) - self.i)
        for _ in range(n):
            self.q[self.i]()
            self.i += 1

    def require(self, name):
        m = self.marks.get(name, 0)
        while self.i < m:
            self.q[self.i]()
            self.i += 1

    def drain(self):
        while self.i < len(self.q):
            self.q[self.i]()
            self.i += 1


def _emit(tc: tile.TileContext, ctx):
    nc = tc.nc

    xT = nc.dram_tensor("xT", [D, S], BF, kind="ExternalInput").ap()
    wqt = nc.dram_tensor("wqt", [D, DSH], BF, kind="ExternalInput").ap()
    wkt = nc.dram_tensor("wkt", [D, DSH], BF, kind="ExternalInput").ap()
    wvt = nc.dram_tensor("wvt", [D, DSH], BF, kind="ExternalInput").ap()
    wot = nc.dram_tensor("wot", [DSH, D], BF, kind="ExternalInput").ap()
    pad0 = nc.dram_tensor("pad0", [128, NKC], F32, kind="ExternalInput").ap()
    tri = nc.dram_tensor("tri", [128, TRI_W], BF, kind="ExternalInput").ap()
    yT = nc.dram_tensor("yT", [D, S], BF, kind="ExternalOutput").ap()

    persist = ctx.enter_context(tc.tile_pool(name="persist", bufs=1))
    sc_pool = ctx.enter_context(tc.tile_pool(name="scps", bufs=2, space="PSUM"))
    ct_pool = ctx.enter_context(tc.tile_pool(name="ctps", bufs=2, space="PSUM"))
    pp_pool = ctx.enter_context(tc.tile_pool(name="ppps", bufs=2, space="PSUM"))
    pu_pool = ctx.enter_context(tc.tile_pool(name="pu", bufs=6))
    work = ctx.enter_context(tc.tile_pool(name="work", bufs=4))
    dpool = ctx.enter_context(tc.tile_pool(name="dram", bufs=1, space="DRAM"))

    xs = persist.tile([128, 8, S], BF)
    wq_s = persist.tile([128, 8, DSH], BF)
    wk_s = persist.tile([128, 8, DSH], BF)
    wv_s = persist.tile([128, 8, DSH], BF)
    wo_s = persist.tile([128, 2, D], BF)
    pad_s = persist.tile([128, NKC], F32)
    tri_s = persist.tile([128, TRI_W], BF)
    qt2 = persist.tile([128, 2, S], BF)
    kt2 = persist.tile([128, 2, S], BF)
    vp = persist.tile([128, NKC, 65 * HPC], BF)
    ctn = persist.tile([128, 2, S], BF)
    ctu = persist.tile([65, 16, 512], F32)    # unnormalized ctx + L, per (h, qt)
    lall0 = persist.tile([8, 512], F32)
    # pair-1 L rows stay on partition 64 (no partition shift on engines):
    # lqa[64, qt, idx, :] holds head (2+idx)'s row-sum for query tile qt.
    lqa = persist.tile([65, 4, 2, 512], F32)
    lqb = persist.tile([65, 2, 512], BF)
    ones65 = persist.tile([65, 64], BF)
    ldram = dpool.tile([8, 512], F32)
    ldram1 = dpool.tile([8, 512], F32)

    wtile = persist.tile([128, 128], BF)
    nc.vector.memset(wtile, 0.001)
    nc.vector.memset(ones65, 1.0)
    nc.gpsimd.memset(vp, 1.0)
    nc.gpsimd.memset(lqa, 1.0)   # rows 0-63 never hold data; keep them finite

    # ---- input DMAs: weights first, x in 512-column waves so the first
    # projections can start after ~1MB instead of ~4MB ----
    xr = xT.rearrange("(c p) s -> p c s", p=128)
    wqr = wqt.rearrange("(c p) j -> p c j", p=128)
    wkr = wkt.rearrange("(c p) j -> p c j", p=128)
    wvr = wvt.rearrange("(c p) j -> p c j", p=128)
    wor = wot.rearrange("(c p) o -> p c o", p=128)
    nc.sync.dma_start(out=pad_s, in_=pad0)
    nc.sync.dma_start(out=tri_s, in_=tri)
    engs = [nc.sync, nc.gpsimd, nc.scalar]
    ei = 0

    def dma_rr(out, in_):
        nonlocal ei
        engs[ei % len(engs)].dma_start(out=out, in_=in_)
        ei += 1

    for c in range(8):
        dma_rr(xs[:, c, 0:512], xr[:, c, 0:512])
        dma_rr(wq_s[:, c, :], wqr[:, c, :])
        dma_rr(wk_s[:, c, :], wkr[:, c, :])
    engs = [nc.sync, nc.gpsimd]
    for c in range(8):
        dma_rr(wv_s[:, c, :], wvr[:, c, :])
        dma_rr(xs[:, c, 512:1024], xr[:, c, 512:1024])
    for w in range(2, 4):
        for c in range(8):
            dma_rr(xs[:, c, 512 * w : 512 * w + 512], xr[:, c, 512 * w : 512 * w + 512])
    for c in range(2):
        dma_rr(wo_s[:, c, :], wor[:, c, :])

    # warm the PE's DVFS p-state while the first x wave streams in: a run
    # of small matmuls on the tri strip into a never-read pp-pool slot.
    wps = pp_pool.tile([128, 512], F32, tag="pp", name="warmps")
    for i in range(96):
        nc.tensor.matmul(
            wps[:, 0:128], wtile, wtile,
            start=True, stop=True, skip_group_check=True,
        )

    filler = Filler()   # pair-0 projections (qk pair0 + v)
    fillerB = Filler()  # pair-1 projections + output projection

    # ---- filler unit builders ------------------------------------------
    def add_qk_output(fl, dht, wsb, dst, st):
        """8 accumulating matmuls + 1 copy-out for one [128,512] q/k slab."""
        state = {}

        def mk(dc):
            def f():
                if dc == 0:
                    state["ps"] = pp_pool.tile([128, 512], F32, tag="pp", name="ppqk")
                ps = state["ps"]
                nc.tensor.matmul(
                    ps,
                    wsb[:, dc, 128 * dht : 128 * dht + 128],
                    xs[:, dc, 512 * st : 512 * st + 512],
                    start=(dc == 0),
                    stop=(dc == 7),
                )
                if dc == 7:
                    sl = dst[:, dht, 512 * st : 512 * st + 512]
                    nc.vector.tensor_copy(out=sl, in_=ps)

            return f

        for dc in range(8):
            fl.add(mk(dc))

    def add_v_chunk(sc):
        """8 accumulating matmuls + pad-mask copy-outs for one V s-chunk."""
        state = {}

        def mk(dc):
            def f():
                if dc == 0:
                    state["ps"] = pp_pool.tile([128, 512], F32, tag="pp", name="ppv")
                ps = state["ps"]
                nc.tensor.matmul(
                    ps[:, 0:DSH],
                    xs[:, dc, 128 * sc : 128 * sc + 128],
                    wv_s[:, dc, :],
                    start=(dc == 0),
                    stop=(dc == 7),
                )
                if dc == 7:
                    vd = vp[:, sc, :].rearrange("p (h u) -> p h u", u=65)
                    nc.vector.tensor_scalar_mul(
                        vd[:, :, 0:64],
                        ps[:, 0:DSH].rearrange("p (h u) -> p h u", u=64),
                        pad_s[:, sc : sc + 1],
                    )
                    nc.vector.tensor_scalar_mul(
                        vd[:, :, 64:65], vd[:, :, 64:65], pad_s[:, sc : sc + 1]
                    )

            return f

        for dc in range(8):
            filler.add(mk(dc))
        filler.mark(f"vs{sc}")

    def add_out_proj(st, deep=False):
        """Output projection for one 512-query tile: 16 matmuls + copies."""
        yr = yT.rearrange("(ot p) s -> ot p s", p=128)
        state = {}

        def mk(ot, c2):
            def f():
                if c2 == 0:
                    pool = sc_pool if (deep and ot % 2 == 0) else pp_pool
                    tg = "slot" if (deep and ot % 2 == 0) else "pp"
                    state[ot] = pool.tile([128, 512], F32, tag=tg, name="ppo")
                ps = state[ot]
                nc.tensor.matmul(
                    ps,
                    wo_s[:, c2, 128 * ot : 128 * ot + 128],
                    ctn[:, c2, 512 * st : 512 * st + 512],
                    start=(c2 == 0),
                    stop=(c2 == 1),
                )
                if c2 == 1:
                    ystg = work.tile([128, 512], BF, tag="y", name="ystg")
                    if ot % 2 == 0:
                        nc.scalar.copy(ystg, ps)
                    else:
                        nc.vector.tensor_copy(out=ystg, in_=ps)
                    eng = nc.sync if ot % 2 == 0 else nc.gpsimd
                    eng.dma_start(
                        out=yr[ot, :, 512 * st : 512 * st + 512], in_=ystg
                    )

            return f

        if deep:
            order = [(0, 0), (1, 0), (2, 0), (3, 0), (0, 1), (1, 1),
                     (4, 0), (2, 1), (5, 0), (3, 1), (6, 0), (4, 1),
                     (7, 0), (5, 1), (6, 1), (7, 1)]
        else:
            order = [(ot, c2) for ot in range(8) for c2 in range(2)]
        for ot, c2 in order:
            fillerB.add(mk(ot, c2))

    # build the projection queues: q/k slabs just-in-time per qt, v chunks
    # interleaved so vp[sc] lands before its first P@V.  Pair-1 slabs live
    # in fillerB so pair-1's attention keeps its own PE filler supply.
    add_qk_output(filler, 0, wq_s, qt2, 0)
    add_qk_output(filler, 0, wk_s, kt2, 0)
    filler.mark("qk00")
    for sc in range(0, 4):
        add_v_chunk(sc)
    add_qk_output(filler, 0, wq_s, qt2, 1)
    add_qk_output(filler, 0, wk_s, kt2, 1)
    filler.mark("qk01")
    for sc in range(4, 8):
        add_v_chunk(sc)
    add_qk_output(filler, 0, wq_s, qt2, 2)
    add_qk_output(filler, 0, wk_s, kt2, 2)
    filler.mark("qk02")
    for sc in range(8, 12):
        add_v_chunk(sc)
    add_qk_output(filler, 0, wq_s, qt2, 3)
    add_qk_output(filler, 0, wk_s, kt2, 3)
    filler.mark("qk03")
    for sc in range(12, 16):
        add_v_chunk(sc)
    # pair-1 processes qts in order (1,2,3,0); scores for qt need K columns
    # [0, 512*(qt+1)), so emit K slabs for all earlier sts before the Q slab.
    add_qk_output(fillerB, 1, wk_s, kt2, 0)
    add_qk_output(fillerB, 1, wk_s, kt2, 1)
    add_qk_output(fillerB, 1, wq_s, qt2, 1)
    fillerB.mark("qk11")
    add_qk_output(fillerB, 1, wk_s, kt2, 2)
    add_qk_output(fillerB, 1, wq_s, qt2, 2)
    fillerB.mark("qk12")
    add_qk_output(fillerB, 1, wk_s, kt2, 3)
    add_qk_output(fillerB, 1, wq_s, qt2, 3)
    fillerB.mark("qk13")
    add_qk_output(fillerB, 1, wq_s, qt2, 0)
    fillerB.mark("qk10")

    # ---- attention -----------------------------------------------------
    p1_post = []

    def run_p1_post():
        while p1_post:
            p1_post.pop(0)()

    kept_ct = {}

    def attention(hp, qt, keep_ct=False):
        fl = filler if hp == 0 else fillerB
        fl.require(f"qk{hp}{qt}")
        Q0 = 512 * qt
        nkc = 4 * qt + 4
        ct_e = ct_pool.tile([65, 512], F32, tag="ct", name="ct_e")
        ct_o = ct_pool.tile([65, 512], F32, tag="ct", name="ct_o")
        he, ho = 2 * hp, 2 * hp + 1

        def emit_pv(kc, pu, co, w):
            # P@V runs one chunk behind scores/exp so every PE instruction
            # only depends on work finished >=1 chunk ago (no in-order
            # stalls; keeps the DVFS p-state high).
            nc.tensor.matmul(
                ct_e[:, co : co + w],
                vp[:, kc, 65 * he : 65 * he + 65], pu[:, 0:w],
                start=(kc == 0), stop=(kc == nkc - 1),
            )
            nc.tensor.matmul(
                ct_o[:, co : co + w],
                vp[:, kc, 65 * ho : 65 * ho + 65], pu[:, 512 : 512 + w],
                start=(kc == 0), stop=(kc == nkc - 1),
            )

        prev = None
        for kc in range(nkc):
            K0 = 128 * kc
            band = K0 >= Q0
            # band tiles only cover their live query range [K0, Q0+512)
            qs = K0 if band else Q0
            w = Q0 + 512 - qs
            co = qs - Q0  # ct column offset
            qe = qt2[0:64, hp, qs : qs + w]
            qo = qt2[64:128, hp, qs : qs + w]
            if hp == 0:
                filler.require(f"vs{kc}")
            # PE slot order: [filler, PV(kc-1), filler, scores(kc), filler]
            # -- every PE instruction has >=1 chunk of semaphore slack, so
            # the in-order queue never stalls and the p-state stays high.
            fl.pump(0 if (hp == 1 and kc < 2) else 2)
            if prev is not None:
                emit_pv(*prev)
            fl.pump(0 if (hp == 1 and kc < 2) else 1)
            # heads stay at fixed 512-col offsets (PSUM bank alignment)
            sc = sc_pool.tile([128, 1024], F32, tag="slot", name="sc")
            nc.tensor.matmul(
                sc[:, 0:w], kt2[0:64, hp, K0 : K0 + 128], qe,
                start=True, stop=True,
            )
            nc.tensor.matmul(
                sc[:, 512 : 512 + w], kt2[64:128, hp, K0 : K0 + 128], qo,
                start=True, stop=True,
            )
            pu = pu_pool.tile([128, 1024], BF, tag="pu", name="pu")
            sc2 = sc.rearrange("p (t f) -> p t f", t=2)[:, :, 0:w]
            pu2 = pu.rearrange("p (t f) -> p t f", t=2)[:, :, 0:w]
            nc.scalar.activation(out=pu2, in_=sc2, func=EXP, scale=0.125)
            if hp == 1 and kc == 0:
                run_p1_post()
            if band:  # causal mask; q starts at K0 so the slice is fixed
                tsl = tri_s[:, 384 : 384 + w]
                nc.vector.tensor_mul(pu[:, 0:w], pu[:, 0:w], tsl)
                nc.vector.tensor_mul(
                    pu[:, 512 : 512 + w], pu[:, 512 : 512 + w], tsl
                )
            prev = (kc, pu, co, w)
            fl.pump(0 if (hp == 1 and kc < 2) else 2)
        emit_pv(*prev)
        for idx, cta in ((0, ct_e), (1, ct_o)):
            hq = (2 * hp + idx) * 4 + qt
            if hp == 1:
                # L row straight from PSUM; split across DVE/ACT lanes
                leng = nc.vector if idx == 0 else nc.scalar
                if leng is nc.scalar:
                    nc.scalar.copy(lqa[64:65, qt, idx, :], cta[64:65, :])
                else:
                    nc.vector.tensor_copy(
                        out=lqa[64:65, qt, idx, :], in_=cta[64:65, :]
                    )
            if idx == 1:
                nc.scalar.copy(ctu[:, hq, :], cta)
            else:
                nc.vector.tensor_copy(out=ctu[:, hq, :], in_=cta)
            if hp == 0:
                nc.sync.dma_start(
                    out=lall0[idx * 4 + qt : idx * 4 + qt + 1, :],
                    in_=ctu[64:65, hq, :],
                )

    def _bcast64(src_row):
        """[1, 512] SBUF row -> [64, 512] tile via partition-broadcast DMA."""
        rlb = work.tile([64, 512], F32, tag="rlb", name="rlb")
        bsrc = bass.AP(
            tensor=src_row.tensor, offset=src_row.offset,
            ap=[[0, 64]] + list(src_row.ap[1:]),
        )
        nc.sync.dma_start(out=rlb, in_=bsrc)
        return rlb

    def norm_recip0():
        nc.vector.tensor_scalar_max(lall0, lall0, 1e-30)
        nc.vector.reciprocal_approx_fast(lall0, lall0)
        nc.sync.dma_start(out=ldram, in_=lall0)

    def norm_apply(hp, qt, idx, rlb, eng, src=None):
        Q0 = 512 * qt
        hq = (2 * hp + idx) * 4 + qt
        in0 = ctu[0:64, hq, :] if src is None else src[0:64, :]
        if idx == 0:
            eng.tensor_mul(ctn[0:64, hp, Q0 : Q0 + 512], in0, rlb)
        else:
            stg = work.tile([64, 512], BF, tag="stg", name="stg")
            eng.tensor_mul(stg, in0, rlb)
            nc.sync.dma_start(out=ctn[64:128, hp, Q0 : Q0 + 512], in_=stg)

    def norm_qt1(qt, last=False):
        """Per-qt normalize for head pair 1.  Mid-run qts: DRAM-bounce
        broadcast deferred into the next attention (GpSimd muls -> no PE or
        DVE stall).  Final qt: bf16 outer-product broadcast on the (idle)
        PE to cut the tail latency.  Custom-DVE ops silently no-op at
        partition base 64 on HW, so max/recip run over all 65 partitions."""
        lq = lqa[:, qt, :, :]
        nc.vector.tensor_scalar_max(lq, lq, 1e-30)
        if last:
            nc.vector.reciprocal_approx_fast(lq, lq)
            nc.scalar.copy(lqb[64:65, :, :], lqa[64:65, qt, :, :])
            rlbs = []
            for idx in (0, 1):
                rlb = pp_pool.tile([64, 512], F32, tag="pp", name="rlbp")
                nc.tensor.matmul(
                    rlb, ones65[64:65, :], lqb[64:65, idx, :],
                    start=True, stop=True,
                )
                rlbs.append(rlb)
            for idx in (1, 0):
                norm_apply(1, qt, idx, rlbs[idx], nc.vector)
        else:
            nc.vector.reciprocal_approx_fast(lq, lq)
            nc.sync.dma_start(
                out=ldram1[2 * qt : 2 * qt + 2, :], in_=lqa[64:65, qt, :, :]
            )

            def phase2(qt=qt):
                for idx in (1, 0):
                    norm_apply(
                        1, qt, idx,
                        _bcast64(ldram1[2 * qt + idx : 2 * qt + idx + 1, :]),
                        nc.vector,
                    )

            p1_post.append(phase2)

    # ---- schedule ------------------------------------------------------
    filler.require("qk00")
    for qt in range(4):
        attention(0, qt)
    # normalize pair 0 while pair 1's attention runs; muls go to GpSimd so
    # the Vector queue stays clear for pair-1 band masks.
    norm_recip0()
    P1_ORDER = (1, 2, 3, 0)
    p0_applies = [(qt, idx, idx * 4 + qt) for qt in P1_ORDER for idx in (0, 1)]

    def emit_p0_norm(k):
        while p0_applies and k > 0:
            qt, idx, hq = p0_applies.pop(0)
            norm_apply(0, qt, idx, _bcast64(ldram[hq : hq + 1, :]), nc.gpsimd)
            k -= 1

    for qt in P1_ORDER:
        emit_p0_norm(2)
        attention(1, qt, keep_ct=(qt == P1_ORDER[-1]))
        emit_p0_norm(2)
        norm_qt1(qt, last=(qt == 0))
        add_out_proj(qt, deep=(qt == 0))
    run_p1_post()
    filler.drain()
    fillerB.drain()


def build_nc():
    nc = bacc.Bacc(
        "TRN2",
        target_bir_lowering=False,
        debug=False,
        enable_asserts=False,
        num_devices=NCORES,
    )
    from contextlib import ExitStack

    with tile.TileContext(nc) as tc:
        with ExitStack() as ctx:
            _emit(tc, ctx)
    nc.compile()
    return nc


def _get_nc():
    if not _NC_CACHE:
        _NC_CACHE.append(build_nc())
    return _NC_CACHE[0]


def make_tri() -> np.ndarray:
    p = np.arange(128)[:, None]
    v = np.arange(TRI_W)[None, :]
    return (p <= v - 384).astype(np.float32).astype(ml_dtypes.bfloat16)


def make_in_maps(x, mask, WQ, WK, WV, WO):
    bf = ml_dtypes.bfloat16
    tri = make_tri()
    in_maps = []
    for c in range(NCORES):
        b, g = c // (NCORES // B), c % (NCORES // B)
        sl = slice(DSH * g, DSH * g + DSH)
        in_maps.append(
            {
                "xT": np.ascontiguousarray(x[b].T).astype(bf),
                "wqt": np.ascontiguousarray(WQ[sl, :].T).astype(bf),
                "wkt": np.ascontiguousarray(WK[sl, :].T).astype(bf),
                "wvt": np.ascontiguousarray(WV[sl, :].T).astype(bf),
                "wot": np.ascontiguousarray(WO[:, sl].T).astype(bf),
                "pad0": np.ascontiguousarray(
                    (mask[b] == 0).astype(np.float32).reshape(NKC, 128).T
                ),
                "tri": tri,
            }
        )
    return in_maps


def assemble(results, x, mask, WV, WO, bO) -> np.ndarray:
    y = np.zeros((B, S, D), np.float32)
    for c in range(NCORES):
        y[c // (NCORES // B)] += results[c]["yT"].T
    y += bO[None, None, :]
    # Rows i < first-unmasked-index are fully masked in the reference; its
    # softmax then degenerates to uniform attention over all positions.
    for b in range(B):
        nz = np.nonzero(mask[b] == 0)[0]
        t = int(nz[0]) if nz.size else S
        if t > 0:
            vbar = x[b].mean(axis=0) @ WV.T
            yfix = vbar @ WO.T + bO
            y[b, :t, :] = yfix
    return y


def kernel(x, mask, WQ, WK, WV, WO, bO) -> np.ndarray:
    x = np.asarray(x, np.float32)
    mask = np.asarray(mask, np.int32)
    WQ = np.asarray(WQ, np.float32)
    WK = np.asarray(WK, np.float32)
    WV = np.asarray(WV, np.float32)
    WO = np.asarray(WO, np.float32)
    bO = np.asarray(bO, np.float32)

    nc = _get_nc()
    in_maps = make_in_maps(x, mask, WQ, WK, WV, WO)
    res = run_bass_kernel_spmd(nc, in_maps, list(range(NCORES)))
    return assemble(res.results, x, mask, WV, WO, bO)
